# Initial kernel scaffold
#
"""LoRA Linear (residual + low-rank path with dropout) on 8 Trainium2 cores.

Math (fp32 reference):
  residual = hidden_states @ W_base.T
  dropped  = hidden_states * dropout_mask / (1 - p)
  out      = residual + ((dropped @ A.T) @ B.T) * scaling

Sharding: data-parallel over the 8192 tokens (8 cores x 1024 tokens);
W_base / A / B replicated. All matmuls run on the PE in float32r (full
fp32 bits, reduced-precision multiply array) which streams at ~1
cycle/row — ~70 TF/s/core vs 19.6 TF/s for plain fp32, with ~2.5e-4
scale-relative error on this problem.

Key constraints this layout honors (measured on HW):
  - DMA is the scarce resource (~358 GB/s/core): W streams exactly ONCE
    (x stays resident in SBUF for all 1024 tokens), everything is
    host-pre-tiled so each DMA reads large contiguous runs.
  - Output DMAs issue from the ACT engine so the SP engine's HWDGE
    stream (all input loads) never blocks on a compute semaphore.
  - The rank-16 LoRA product accumulates into the same PSUM tile as
    the residual matmul (K=16 matmul, start=False), so the add is free.
  - 1/(1-p) is folded into A, `scaling` into B on the host.
"""

import numpy as np

P = 128
D_IN = 4096
D_OUT = 4096
BATCH, SEQ = 4, 2048
TOK = BATCH * SEQ  # 8192
NCORES = 8
T = TOK // NCORES  # 1024 tokens per core, all resident
KT = D_IN // P  # 32 k-tiles
NO = 256  # out-dim chunk width
OC = D_OUT // NO  # 16
TT = T // P  # 8 token tiles
NP_ = 256  # xa-matmul free-dim chunk (>=192 keeps f32r on the fast path)
PH = T // NP_  # 4
R = 16
DPIECE = 8  # k-tiles per WT/xT DMA piece
DROP_P = 0.05
SCALING = 32.0 / 16.0

_PROGRAM_CACHE = {}


def _build_program():
    from concourse import bacc
    import concourse.mybir as mybir
    import concourse.tile as tile

    f32 = mybir.dt.float32
    f32r = mybir.dt.float32r
    u8 = mybir.dt.uint8

    nc = bacc.Bacc("TRN2", target_bir_lowering=False)
    xT_d = nc.dram_tensor("xT", [KT, P, T], f32r, kind="ExternalInput")
    mT_d = nc.dram_tensor("mT", [KT, P, T], u8, kind="ExternalInput")
    WT_d = nc.dram_tensor("WT", [OC, KT, P, NO], f32r, kind="ExternalInput")
    AT_d = nc.dram_tensor("AT", [P, KT, R], f32r, kind="ExternalInput")
    BT_d = nc.dram_tensor("BT", [OC, R, NO], f32r, kind="ExternalInput")
    out_d = nc.dram_tensor("out", [OC, TT, P, NO], f32, kind="ExternalOutput")

    with tile.TileContext(nc) as tc:
        with (
            tc.tile_pool(name="at", bufs=4) as atpool,
            tc.tile_pool(name="xt", bufs=1) as xtpool,
            tc.tile_pool(name="wt", bufs=2) as wtpool,
            tc.tile_pool(name="bt", bufs=2) as btpool,
            tc.tile_pool(name="m", bufs=3) as mpool,
            tc.tile_pool(name="d", bufs=2) as dpool,
            tc.tile_pool(name="xa", bufs=1) as xapool,
            tc.tile_pool(name="o", bufs=2) as opool,
            tc.tile_pool(name="ps_xa", bufs=4, space="PSUM") as ps_xa,
            tc.tile_pool(name="ps_mm", bufs=4, space="PSUM") as ps_mm,
        ):
            # resident x (f32r view; prologue reads it as f32 via bitcast);
            # pieces are loaded inside the prologue k-loop so the mask DMAs
            # interleave with them instead of queueing behind all of x
            xT_t = xtpool.tile([P, KT, T], f32r, tag="xT")

            WT_pre = {}

            def preload_wt(oc):
                WT_t = wtpool.tile([P, KT, NO], f32r, tag="WT", name=f"WT{oc}")
                for k0 in range(0, KT, DPIECE):
                    nc.sync.dma_start(
                        WT_t[:, k0 : k0 + DPIECE],
                        WT_d[oc, k0 : k0 + DPIECE].rearrange("k p o -> p k o"),
                    )
                BT_t = btpool.tile([R, NO], f32r, tag="BT", name=f"BT{oc}")
                nc.sync.dma_start(BT_t[:], BT_d[oc])
                WT_pre[oc] = (WT_t, BT_t)

            # ---- LoRA first stage: xaT[r, t] = (A/(1-p)) @ (x * mask)
            xa_ps = [
                ps_xa.tile([R, NP_], f32, tag="xa", name=f"xa_ps{h}")
                for h in range(PH)
            ]
            for k in range(KT):
                if k % DPIECE == 0:
                    nc.sync.dma_start(
                        xT_t[:, k : k + DPIECE],
                        xT_d[k : k + DPIECE].rearrange("k p t -> p k t"),
                    )
                if k == DPIECE:
                    preload_wt(0)
                at_t = atpool.tile([P, R], f32r, tag="AT", name=f"AT{k}")
                nc.sync.dma_start(at_t[:], AT_d[:, k])
                m_t = mpool.tile([P, T], u8, tag="m", name=f"m{k}")
                nc.sync.dma_start(m_t[:], mT_d[k])
                for g in range(2):
                    gs = slice(g * (T // 2), (g + 1) * (T // 2))
                    d_t = dpool.tile([P, T // 2], f32r, tag="d", name=f"d{k}_{g}")
                    nc.vector.tensor_tensor(
                        d_t[:], xT_t[:, k, gs].bitcast(f32), m_t[:, gs],
                        mybir.AluOpType.mult,
                    )
                    for h in range(PH // 2):
                        nc.tensor.matmul(
                            xa_ps[g * (PH // 2) + h][:],
                            at_t[:],
                            d_t[:, h * NP_ : (h + 1) * NP_],
                            start=(k == 0),
                            stop=(k == KT - 1),
                        )
            xaT_t = xapool.tile([R, T], f32r, tag="xaT")
            for h in range(PH):
                nc.vector.tensor_copy(
                    xaT_t[:, h * NP_ : (h + 1) * NP_], xa_ps[h][:]
                )

            # ---- main matmul + lora accumulate + drain
            for oc in range(OC):
                if oc in WT_pre:
                    WT_t, BT_t = WT_pre[oc]
                else:
                    WT_t = wtpool.tile([P, KT, NO], f32r, tag="WT", name=f"WT{oc}")
                    for k0 in range(0, KT, DPIECE):
                        nc.sync.dma_start(
                            WT_t[:, k0 : k0 + DPIECE],
                            WT_d[oc, k0 : k0 + DPIECE].rearrange("k p o -> p k o"),
                        )
                    BT_t = btpool.tile([R, NO], f32r, tag="BT", name=f"BT{oc}")
                    nc.sync.dma_start(BT_t[:], BT_d[oc])

                for tt in range(TT):
                    ps = ps_mm.tile([P, NO], f32, tag="ps", name=f"ps{oc}_{tt}")
                    for k in range(KT):
                        nc.tensor.matmul(
                            ps[:],
                            xT_t[:, k, tt * P : (tt + 1) * P],
                            WT_t[:, k],
                            start=(k == 0),
                            stop=False,
                        )
                    nc.tensor.matmul(
                        ps[:],
                        xaT_t[:, tt * P : (tt + 1) * P],
                        BT_t[:],
                        start=False,
                        stop=True,
                    )
                    o_t = opool.tile([P, NO], f32, tag="o", name=f"o{oc}_{tt}")
                    nc.vector.tensor_copy(o_t[:], ps[:])
                    nc.scalar.dma_start(out_d[oc, tt], o_t[:])

    nc.finalize()
    return nc


def _get_program():
    if "nc" not in _PROGRAM_CACHE:
        _PROGRAM_CACHE["nc"] = _build_program()
    return _PROGRAM_CACHE["nc"]


def kernel(hidden_states, W_base, A, B, dropout_mask):
    from concourse.bass_utils import run_bass_kernel_spmd

    hs = np.ascontiguousarray(np.asarray(hidden_states, dtype=np.float32)).reshape(
        TOK, D_IN
    )
    mask = np.asarray(dropout_mask).reshape(TOK, D_IN)
    W = np.asarray(W_base, dtype=np.float32)
    A_ = np.asarray(A, dtype=np.float32)
    B_ = np.asarray(B, dtype=np.float32)

    # Shared, pre-tiled weight layouts (contiguous per device DMA):
    #   WT[oc, k, p, o] = W[oc*NO+o, k*P+p]
    WT = np.ascontiguousarray(
        W.reshape(OC, NO, KT, P).transpose(0, 2, 3, 1).astype(np.float32)
    )
    #   AT[p, k, r] = A[r, k*P+p] / (1-p)
    AT = np.ascontiguousarray(
        A_.T.reshape(KT, P, R).transpose(1, 0, 2) * np.float32(1.0 / (1.0 - DROP_P))
    ).astype(np.float32)
    #   BT[oc, r, o] = B[oc*NO+o, r] * scaling
    BT = np.ascontiguousarray(
        B_.T.reshape(R, OC, NO).transpose(1, 0, 2) * np.float32(SCALING)
    ).astype(np.float32)

    in_maps = []
    for c in range(NCORES):
        sl = slice(c * T, (c + 1) * T)
        #   xT[k, p, t] = x[c*T + t, k*P+p]
        xT = np.ascontiguousarray(hs[sl].T).reshape(KT, P, T)
        mT = np.ascontiguousarray(mask[sl].T).astype(np.uint8).reshape(KT, P, T)
        in_maps.append({"xT": xT, "mT": mT, "WT": WT, "AT": AT, "BT": BT})

    nc = _get_program()
    res = run_bass_kernel_spmd(nc, in_maps, core_ids=list(range(NCORES)))
    _PROGRAM_CACHE["last_results"] = res

    # out_dev[oc, g, p, o] = out[g*P+p, oc*NO+o]  (per core)
    parts = []
    for c in range(NCORES):
        od = res.results[c]["out"]  # [OC, TT, P, NO]
        parts.append(od.transpose(1, 2, 0, 3).reshape(T, D_OUT))
    out = np.concatenate(parts, axis=0)
    return out.reshape(BATCH, SEQ, D_OUT).astype(np.float32)



# revision 10
# speedup vs baseline: 1.2048x; 1.2048x over previous
"""LoRA Linear (residual + low-rank path with dropout) on 8 Trainium2 cores.

Math (fp32 reference):
  residual = hidden_states @ W_base.T
  dropped  = hidden_states * dropout_mask / (1 - p)
  out      = residual + ((dropped @ A.T) @ B.T) * scaling

Sharding: data-parallel over the 8192 tokens (8 cores x 1024 tokens);
W_base / A / B replicated.  All matmuls run on the PE in float32r.

Layout strategy (v4): the stationary operand is a W o-tile [128k x 128o]
and the moving operand is a 512-token slice of x [128k x 512t], PSUM
out = [128o x 512t].  At N=512 the per-matmul stream time (213 ns) has
slack over its f32r LDWEIGHTS (~190 ns with chase), and each PSUM bank
receives 33 back-to-back matmuls (k-loop + LoRA accumulate) before the
bank switches -- alternating banks per matmul breaks the PE's
LDWEIGHTS chase and serializes LDW->MM at ~272 ns/MM (measured).

The LoRA accumulate is zero-padded to K=128 (B rows 16..127 = 0, xa
rows 16..127 memset once) so every PE instruction is a homogeneous
[128x128] x [128x512] matmul -- a K=16 matmul at the group boundary
measured ~+400 ns per group (2x64 groups -> ~26 us).

Keeping x resident (128 KB/part) and streaming W in 16 KB/part o-tiles
keeps SBUF under the ~208 KB/part budget while W still streams exactly
once from HBM.  1/(1-p) is folded into A, `scaling` into B on the host.

Queues: x/mask stream on the SP HWDGE ring in 4-k-tile pieces; W0 on
the ACT HWDGE ring; the remaining W tiles and per-o-tile B tiles on the
Pool/GpSimd SWDGE ring; output DMAs on the ACT ring; DVE does the
mask-multiplies and PSUM drains.  The first three o-tiles' k-runs are
interleaved piece-by-piece with LoRA stage 1 so the PE has work while
x streams in.
"""

import numpy as np

P = 128
D_IN = 4096
D_OUT = 4096
BATCH, SEQ = 4, 2048
TOK = BATCH * SEQ  # 8192
NCORES = 8
T = TOK // NCORES  # 1024 tokens per core, all resident
KT = D_IN // P  # 32 k-tiles
OT = D_OUT // P  # 32 out-tiles of 128
TH = 2  # moving-dim halves (512 tokens each)
NF = T // TH  # 512 moving free dim
KP = 8  # x/mask DMA pieces (4 k-tiles each)
KPK = KT // KP
R = 16
DROP_P = 0.05
SCALING = 32.0 / 16.0
N_EARLY = 3  # o-tiles interleaved with the prologue
W_BUFS = 3
BT_BUFS = 4

_PROGRAM_CACHE = {}


def _build_program():
    from concourse import bacc
    import concourse.mybir as mybir
    import concourse.tile as tile

    f32 = mybir.dt.float32
    f32r = mybir.dt.float32r
    u8 = mybir.dt.uint8

    nc = bacc.Bacc("TRN2", target_bir_lowering=False)
    xT_d = nc.dram_tensor("xT", [P, KT, T], f32r, kind="ExternalInput")
    mT_d = nc.dram_tensor("mT", [P, KT, T], u8, kind="ExternalInput")
    WT_d = nc.dram_tensor("WT", [OT, P, KT, P], f32r, kind="ExternalInput")
    AT_d = nc.dram_tensor("AT", [P, KT, R], f32r, kind="ExternalInput")
    BT_d = nc.dram_tensor("BT", [OT, P, P], f32r, kind="ExternalInput")
    out_d = nc.dram_tensor("out", [OT, TH, P, NF], f32, kind="ExternalOutput")

    with tile.TileContext(nc) as tc:
        with (
            tc.tile_pool(name="xt", bufs=1) as xtpool,
            tc.tile_pool(name="at", bufs=1) as atpool,
            tc.tile_pool(name="bt", bufs=BT_BUFS) as btpool,
            tc.tile_pool(name="wt", bufs=W_BUFS) as wtpool,
            tc.tile_pool(name="m", bufs=2) as mpool,
            tc.tile_pool(name="d", bufs=2) as dpool,
            tc.tile_pool(name="xa", bufs=1) as xapool,
            tc.tile_pool(name="z", bufs=1) as zpool,
            tc.tile_pool(name="o", bufs=2) as opool,
            tc.tile_pool(name="ps_xa", bufs=2, space="PSUM") as ps_xa,
            tc.tile_pool(name="ps_mm", bufs=2 * N_EARLY, space="PSUM") as ps_mm,
        ):
            xT_t = xtpool.tile([P, KT, T], f32r, tag="xT")
            AT_t = atpool.tile([P, KT, R], f32r, tag="AT")
            nc.sync.dma_start(AT_t[:], AT_d[:])

            WT_ts = {}

            def load_w(ot, queue):
                WT_t = wtpool.tile([P, KT, P], f32r, tag="WT", name=f"WT{ot}")
                queue.dma_start(WT_t[:], WT_d[ot])
                WT_ts[ot] = WT_t

            BT_ts = {}

            def load_b(ot):
                BT_t = btpool.tile([P, P], f32r, tag="BT", name=f"BT{ot}")
                nc.gpsimd.dma_start(BT_t[:], BT_d[ot])
                BT_ts[ot] = BT_t

            # early W tiles: one per DMA ring so the SP ring carries only
            # the x/mask stream during startup
            load_w(0, nc.scalar)
            load_w(1, nc.gpsimd)
            load_w(2, nc.gpsimd)

            # xa, zero-padded to K=128 so the LoRA accumulate is a
            # homogeneous [128x128]x[128x512] matmul
            xaT_t = xapool.tile([P, T], f32r, tag="xaT")
            z_t = zpool.tile([P, T], f32, tag="z")
            nc.vector.memset(z_t[:], 0.0)
            nc.vector.tensor_copy(xaT_t[:], z_t[:])
            for ot in range(BT_BUFS):
                load_b(ot)

            xa_ps = [
                ps_xa.tile([R, NF], f32, tag="xa", name=f"xa_ps{h}")
                for h in range(TH)
            ]
            ps_early = {}
            for ot in range(N_EARLY):
                for th in range(TH):
                    ps_early[(ot, th)] = ps_mm.tile(
                        [P, NF], f32, tag="ps", name=f"ps{ot}_{th}"
                    )

            # ---- prologue: stream x/mask pieces; per piece run the first
            # three o-tiles' partial k-accumulation and LoRA stage 1
            # (xa += A.T @ (x*mask)) so the PE has work as x arrives.
            for kp in range(KP):
                ks = slice(kp * KPK, (kp + 1) * KPK)
                nc.sync.dma_start(xT_t[:, ks], xT_d[:, ks])
                m_t = mpool.tile([P, KPK, T], u8, tag="m", name=f"m{kp}")
                nc.sync.dma_start(m_t[:], mT_d[:, ks])
                for ot in range(N_EARLY):
                    for th in range(TH):
                        ts = slice(th * NF, (th + 1) * NF)
                        for k in range(kp * KPK, (kp + 1) * KPK):
                            nc.tensor.matmul(
                                ps_early[(ot, th)][:],
                                WT_ts[ot][:, k],
                                xT_t[:, k, ts],
                                start=(k == 0), stop=False,
                            )
                for th in range(TH):
                    ts = slice(th * NF, (th + 1) * NF)
                    for k in range(kp * KPK, (kp + 1) * KPK):
                        d_t = dpool.tile([P, NF], f32r, tag="d", name=f"d{k}_{th}")
                        nc.vector.tensor_tensor(
                            d_t[:], xT_t[:, k, ts].bitcast(f32),
                            m_t[:, k - kp * KPK, ts], mybir.AluOpType.mult,
                        )
                        nc.tensor.matmul(
                            xa_ps[th][:], AT_t[:, k], d_t[:],
                            start=(k == 0), stop=(k == KT - 1),
                        )

            for th in range(TH):
                nc.vector.tensor_copy(
                    xaT_t[:R, th * NF : (th + 1) * NF], xa_ps[th][:]
                )

            # ---- main loop: remaining o-tiles + LoRA accumulate + drain
            for ot in range(OT):
                if ot + BT_BUFS < OT:
                    load_b(ot + BT_BUFS)
                nxt = ot + W_BUFS
                if nxt < OT and nxt not in WT_ts:
                    load_w(nxt, nc.gpsimd)
                WT_t = WT_ts[ot]
                if ot < N_EARLY:
                    ps = [ps_early[(ot, th)] for th in range(TH)]
                else:
                    ps = [
                        ps_mm.tile([P, NF], f32, tag="ps", name=f"ps{ot}_{th}")
                        for th in range(TH)
                    ]
                for th in range(TH):
                    ts = slice(th * NF, (th + 1) * NF)
                    if ot >= N_EARLY:
                        for k in range(KT):
                            nc.tensor.matmul(
                                ps[th][:], WT_t[:, k], xT_t[:, k, ts],
                                start=(k == 0), stop=False,
                            )
                    nc.tensor.matmul(
                        ps[th][:], BT_ts[ot][:], xaT_t[:, ts],
                        start=False, stop=True,
                    )
                    o_t = opool.tile([P, NF], f32, tag="o", name=f"o{ot}_{th}")
                    nc.vector.tensor_copy(o_t[:], ps[th][:])
                    nc.scalar.dma_start(out_d[ot, th], o_t[:])

    nc.finalize()
    return nc


def _get_program():
    if "nc" not in _PROGRAM_CACHE:
        _PROGRAM_CACHE["nc"] = _build_program()
    return _PROGRAM_CACHE["nc"]


def kernel(hidden_states, W_base, A, B, dropout_mask):
    from concourse.bass_utils import run_bass_kernel_spmd

    hs = np.ascontiguousarray(np.asarray(hidden_states, dtype=np.float32)).reshape(
        TOK, D_IN
    )
    mask = np.asarray(dropout_mask).reshape(TOK, D_IN)
    W = np.asarray(W_base, dtype=np.float32)
    A_ = np.asarray(A, dtype=np.float32)
    B_ = np.asarray(B, dtype=np.float32)

    # Shared, pre-tiled weight layouts (fully contiguous per-partition DMA):
    #   WT[ot, p, k, o] = W[ot*P+o, k*P+p]
    WT = np.ascontiguousarray(
        W.reshape(OT, P, KT, P).transpose(0, 3, 2, 1)
    ).astype(np.float32)
    #   AT[p, k, r] = A[r, k*P+p] / (1-p)
    AT = np.ascontiguousarray(
        A_.T.reshape(KT, P, R).transpose(1, 0, 2) * np.float32(1.0 / (1.0 - DROP_P))
    ).astype(np.float32)
    #   BT[ot, r, o] = B[ot*P+o, r] * scaling, zero-padded to r=128
    BT = np.zeros((OT, P, P), dtype=np.float32)
    BT[:, :R, :] = B_.T.reshape(R, OT, P).transpose(1, 0, 2) * np.float32(SCALING)

    in_maps = []
    for c in range(NCORES):
        sl = slice(c * T, (c + 1) * T)
        #   xT[p, k, t] = x[c*T + t, k*P+p]
        xT = np.ascontiguousarray(
            hs[sl].T.reshape(KT, P, T).transpose(1, 0, 2)
        )
        mT = np.ascontiguousarray(
            mask[sl].T.reshape(KT, P, T).transpose(1, 0, 2)
        ).astype(np.uint8)
        in_maps.append({"xT": xT, "mT": mT, "WT": WT, "AT": AT, "BT": BT})

    nc = _get_program()
    res = run_bass_kernel_spmd(nc, in_maps, core_ids=list(range(NCORES)))
    _PROGRAM_CACHE["last_results"] = res

    # out_dev[ot, th, o, t] = out[th*NF+t, ot*P+o]  (per core)
    parts = []
    for c in range(NCORES):
        od = res.results[c]["out"]  # [OT, TH, P, NF]
        parts.append(od.transpose(1, 3, 0, 2).reshape(T, D_OUT))
    out = np.concatenate(parts, axis=0)
    return out.reshape(BATCH, SEQ, D_OUT).astype(np.float32)
